# revision 6
# baseline (speedup 1.0000x reference)
"""ColorHistogramLoss Trainium2 kernel (8 NeuronCores, data-parallel).

Strategy: shard batch (32 -> 4 per core). Each core streams its 25MB of
pixels through SBUF in [128, 2048] plane-tiles, computes HSV per pixel on
VectorE (fp32), and produces cumulative histogram-edge counts
C(e) = #{x < e} for 9 edges x 3 components x (real, fake) via fused
tensor_scalar(is_lt, accum_out=...) ops.  Per-(iteration, edge)
per-partition counts are DMA'd out ([8*128, 32] per core); the host sums
partitions/cores, differences cumulative counts into 10-bin histograms and
computes the scalar loss.  All on-device count arithmetic is exact in f32.
"""

import sys

if "/opt/trn_rl_repo" not in sys.path:
    sys.path.insert(0, "/opt/trn_rl_repo")

import numpy as np

from concourse import bacc, mybir, tile
from concourse import bass_utils

# ---- problem constants (hardcoded; kernel.py must be self-contained) ----
B, C, H, W = 32, 3, 512, 512
NCORES = 8
BPC = B // NCORES            # batches per core
P, F = 128, 2048             # SBUF tile: one [512,512] plane = [128, 2048]
NITER = 2 * BPC              # 4 real + 4 fake plane-triple iterations
NEDGE = 27                   # 9 edges x (hue, sat, val)
ACCW = 32                    # padded accumulator width
NPIX = B * H * W             # pixels per full histogram
ALPHA, BETA, GAMMA = 0.3, 0.4, 0.4

AF = mybir.AluOpType
F32 = mybir.dt.float32

LAST_EXEC_NS = None
_CACHE = {}


def _build():
    nc = bacc.Bacc(
        "TRN2", target_bir_lowering=False, debug=False, num_devices=NCORES
    )
    xr = nc.dram_tensor("x_real", [BPC * C * P, F], F32, kind="ExternalInput").ap()
    xf = nc.dram_tensor("x_fake", [BPC * C * P, F], F32, kind="ExternalInput").ap()
    out = nc.dram_tensor("out", [NITER * P, ACCW], F32, kind="ExternalOutput").ap()

    with tile.TileContext(nc) as tc:
        with tc.tile_pool(name="main", bufs=2) as io_pool, tc.tile_pool(
            name="tmp", bufs=1
        ) as tmp_pool:
            for it in range(NITER):
                src = xr if it < BPC else xf
                bi = it % BPC

                def plane(c):
                    q = bi * C + c
                    return src[q * P : (q + 1) * P, :]

                r = io_pool.tile([P, F], F32, tag="r")
                g = io_pool.tile([P, F], F32, tag="g")
                bl = io_pool.tile([P, F], F32, tag="bl")
                nc.sync.dma_start(r[:], plane(0))
                nc.sync.dma_start(g[:], plane(1))
                nc.sync.dma_start(bl[:], plane(2))

                t = [
                    tmp_pool.tile([P, F], F32, tag=f"t{i}", name=f"t{i}")
                    for i in range(10)
                ]
                V = nc.vector

                m1, mx = t[0], t[1]
                V.tensor_tensor(m1[:], r[:], g[:], AF.max)
                V.tensor_tensor(mx[:], m1[:], bl[:], AF.max)
                n1, mn = t[2], t[3]
                V.tensor_tensor(n1[:], r[:], g[:], AF.min)
                V.tensor_tensor(mn[:], n1[:], bl[:], AF.min)
                d = t[2]
                V.tensor_tensor(d[:], mx[:], mn[:], AF.subtract)
                rd = t[3]
                V.reciprocal_approx_fast(rd[:], d[:])
                u = t[0]
                V.tensor_tensor(u[:], g[:], bl[:], AF.subtract)
                v = t[4]
                V.tensor_tensor(v[:], bl[:], r[:], AF.subtract)
                w = t[5]
                V.tensor_tensor(w[:], r[:], g[:], AF.subtract)
                hA = t[6]
                V.tensor_tensor(hA[:], u[:], rd[:], AF.mult)
                hneg = t[7]
                V.tensor_scalar(hneg[:], hA[:], 0.0, None, AF.is_lt)
                hA2 = t[8]
                # hA2 = 6*hneg + hA   (mod-6 fix: hue6 in [-1,1) -> [0,6))
                V.scalar_tensor_tensor(hA2[:], hneg[:], 6.0, hA[:], AF.mult, AF.add)
                hB = t[7]
                V.tensor_tensor(hB[:], v[:], rd[:], AF.mult)
                hC = t[6]
                V.tensor_tensor(hC[:], w[:], rd[:], AF.mult)
                mb = t[0]
                V.tensor_tensor(mb[:], mx[:], bl[:], AF.is_equal)
                mg = t[4]
                V.tensor_tensor(mg[:], mx[:], g[:], AF.is_equal)
                nmg = t[5]
                # nmg = (mb - 1) * mg = -[mg & !mb]
                V.scalar_tensor_tensor(nmg[:], mb[:], 1.0, mg[:], AF.subtract, AF.mult)
                q1 = t[9]
                V.tensor_tensor(q1[:], hB[:], hA2[:], AF.subtract)
                q2n = t[7]
                # q2n = (q1 + 2) * nmg  == -(hB - hA2 + 2)*mg'
                V.scalar_tensor_tensor(q2n[:], q1[:], 2.0, nmg[:], AF.add, AF.mult)
                h1 = t[9]
                V.tensor_tensor(h1[:], hA2[:], q2n[:], AF.subtract)
                q3 = t[4]
                V.tensor_tensor(q3[:], hC[:], hA2[:], AF.subtract)
                q4 = t[6]
                V.scalar_tensor_tensor(q4[:], q3[:], 4.0, mb[:], AF.add, AF.mult)
                hue6 = t[8]
                V.tensor_tensor(hue6[:], h1[:], q4[:], AF.add)

                acc = io_pool.tile([P, ACCW], F32, tag="acc")
                scr = t[5]
                for k in range(1, 10):
                    V.tensor_scalar(
                        scr[:], hue6[:], 0.6 * k, None, AF.is_lt, AF.add,
                        accum_out=acc[:, k - 1 : k],
                    )
                for k in range(1, 10):
                    # d < 0.1k * mx  <=>  sat = d/mx < 0.1k
                    V.scalar_tensor_tensor(
                        scr[:], mx[:], 0.1 * k, d[:], AF.mult, AF.is_gt,
                        accum_out=acc[:, 9 + k - 1 : 9 + k],
                    )
                for k in range(1, 10):
                    V.tensor_scalar(
                        scr[:], mx[:], 0.1 * k, None, AF.is_lt, AF.add,
                        accum_out=acc[:, 18 + k - 1 : 18 + k],
                    )
                nc.sync.dma_start(
                    out[it * P : (it + 1) * P, :NEDGE], acc[:, :NEDGE]
                )

    nc.compile()
    return nc


def _register_ntff_hook():
    """Register the axon NTFF profiling hook (the container's antenv stub
    lacks axon_hooks, so trn_boot's registration was skipped). Also keep
    profile artifacts local instead of uploading to a share."""
    import types

    import antenv

    if "antenv.axon_hooks" not in sys.modules:
        mod = types.ModuleType("antenv.axon_hooks")
        holder = [None]
        mod.set_axon_ntff_profile_hook = lambda h: holder.__setitem__(0, h)
        mod.get_axon_ntff_profile_hook = lambda: holder[0]
        sys.modules["antenv.axon_hooks"] = mod
        antenv.axon_hooks = mod
    from antenv import axon_hooks

    if axon_hooks.get_axon_ntff_profile_hook() is None:
        from trn_agent_boot.trn_boot import _ntff_profile_via_ctypes

        axon_hooks.set_axon_ntff_profile_hook(
            _ntff_profile_via_ctypes("/opt/axon/libaxon_pjrt.so")
        )
    bass_utils.upload_artifacts = lambda tmpdir: tmpdir


def _get_nc():
    if "nc" not in _CACHE:
        _CACHE["nc"] = _build()
    return _CACHE["nc"]


def kernel(x_real: np.ndarray, x_fake: np.ndarray) -> np.ndarray:
    global LAST_EXEC_NS
    nc = _get_nc()

    in_maps = []
    for c in range(NCORES):
        sl = slice(c * BPC, (c + 1) * BPC)
        in_maps.append(
            {
                "x_real": np.ascontiguousarray(x_real[sl]).reshape(BPC * C * P, F),
                "x_fake": np.ascontiguousarray(x_fake[sl]).reshape(BPC * C * P, F),
            }
        )

    import os

    trace = bool(int(os.environ.get("KERNEL_TRACE", "0")))
    if trace:
        _register_ntff_hook()
    res = bass_utils.run_bass_kernel_spmd(
        nc, in_maps, core_ids=list(range(NCORES)), trace=trace
    )
    LAST_EXEC_NS = res.exec_time_ns
    _CACHE["last_res"] = res

    # C_lt[t, comp, k] = #{x < e_{k+1}} summed over cores/partitions
    C_lt = np.zeros((2, 3, 9), np.float64)
    for core_out in res.results:
        o = np.asarray(core_out["out"]).reshape(NITER, P, ACCW)
        s = o[:, :, :NEDGE].sum(axis=1)          # [NITER, 27]
        C_lt[0] += s[:BPC].sum(axis=0).reshape(3, 9)
        C_lt[1] += s[BPC:].sum(axis=0).reshape(3, 9)

    hist = np.zeros((2, 3, 10), np.float64)
    hist[:, :, 0] = C_lt[:, :, 0]
    hist[:, :, 1:9] = C_lt[:, :, 1:] - C_lt[:, :, :-1]
    hist[:, :, 9] = NPIX - C_lt[:, :, 8]

    dmean = np.abs(hist[0] - hist[1]).mean(axis=1)   # [3] = h, s, v
    loss = ALPHA * dmean[0] + BETA * dmean[1] + GAMMA * dmean[2]
    return np.asarray(loss, dtype=np.float32)


# revision 15
# speedup vs baseline: 1.6586x; 1.6586x over previous
"""ColorHistogramLoss Trainium2 kernel (8 NeuronCores, data-parallel).

Strategy: shard batch (32 -> 4 per core). Each core streams its 25MB of
pixels through SBUF in [128, 2048] plane-tiles, computes HSV per pixel on
VectorE (fp32), and produces cumulative histogram-edge counts
C(e) = #{x < e} for 9 edges x 3 components x (real, fake) via fused
tensor_scalar(is_lt, accum_out=...) ops.  Per-(iteration, edge)
per-partition counts are DMA'd out ([8*128, 32] per core); the host sums
partitions/cores, differences cumulative counts into 10-bin histograms and
computes the scalar loss.  All on-device count arithmetic is exact in f32.
"""

import sys

if "/opt/trn_rl_repo" not in sys.path:
    sys.path.insert(0, "/opt/trn_rl_repo")

import numpy as np

from concourse import bacc, mybir, tile
from concourse import bass_utils

# ---- problem constants (hardcoded; kernel.py must be self-contained) ----
B, C, H, W = 32, 3, 512, 512
NCORES = 8
BPC = B // NCORES            # batches per core
P, F = 128, 2048             # SBUF tile: one [512,512] plane = [128, 2048]
NITER = 2 * BPC              # 4 real + 4 fake plane-triple iterations
NEDGE = 27                   # 9 edges x (hue, sat, val)
ACCW = 32                    # padded accumulator width
NPIX = B * H * W             # pixels per full histogram
ALPHA, BETA, GAMMA = 0.3, 0.4, 0.4

AF = mybir.AluOpType
F32 = mybir.dt.float32

LAST_EXEC_NS = None
_CACHE = {}


def _build():
    nc = bacc.Bacc(
        "TRN2", target_bir_lowering=False, debug=False, num_devices=NCORES
    )
    xr = nc.dram_tensor("x_real", [BPC * C * P, F], F32, kind="ExternalInput").ap()
    xf = nc.dram_tensor("x_fake", [BPC * C * P, F], F32, kind="ExternalInput").ap()
    out = nc.dram_tensor("out", [NITER * P, ACCW], F32, kind="ExternalOutput").ap()

    with tile.TileContext(nc) as tc:
        with tc.tile_pool(name="main", bufs=2) as io_pool, tc.tile_pool(
            name="tmp", bufs=1
        ) as tmp_pool:
            # per-edge bias tiles for ScalarE Sign activations
            ebias, hbias = [], []
            for k in range(1, 10):
                bt = tmp_pool.tile([P, 1], F32, tag=f"eb{k}", name=f"eb{k}")
                nc.gpsimd.memset(bt[:], -(0.1 * k))
                ebias.append(bt)
                ht = tmp_pool.tile([P, 1], F32, tag=f"hb{k}", name=f"hb{k}")
                nc.gpsimd.memset(ht[:], -(0.6 * k))
                hbias.append(ht)
            for it in range(NITER):
                src = xr if it < BPC else xf
                bi = it % BPC

                def plane(c):
                    q = bi * C + c
                    return src[q * P : (q + 1) * P, :]

                r = io_pool.tile([P, F], F32, tag="r")
                g = io_pool.tile([P, F], F32, tag="g")
                bl = io_pool.tile([P, F], F32, tag="bl")
                nc.sync.dma_start(r[:], plane(0))
                nc.sync.dma_start(g[:], plane(1))
                nc.sync.dma_start(bl[:], plane(2))

                t = [
                    tmp_pool.tile([P, F], F32, tag=f"t{i}", name=f"t{i}")
                    for i in range(12)
                ]
                V = nc.vector

                m1, mx = t[0], t[1]
                V.tensor_tensor(m1[:], r[:], g[:], AF.max)
                V.tensor_tensor(mx[:], m1[:], bl[:], AF.max)
                n1, mn = t[2], t[3]
                V.tensor_tensor(n1[:], r[:], g[:], AF.min)
                V.tensor_tensor(mn[:], n1[:], bl[:], AF.min)
                d = t[2]
                V.tensor_tensor(d[:], mx[:], mn[:], AF.subtract)
                rd = t[3]
                V.reciprocal_approx_fast(rd[:], d[:])
                rmx = t[10]
                V.reciprocal_approx_fast(rmx[:], mx[:])
                sat = t[11]
                V.tensor_tensor(sat[:], d[:], rmx[:], AF.mult)
                u = t[0]
                V.tensor_tensor(u[:], g[:], bl[:], AF.subtract)
                v = t[4]
                V.tensor_tensor(v[:], bl[:], r[:], AF.subtract)
                w = t[5]
                V.tensor_tensor(w[:], r[:], g[:], AF.subtract)
                hA = t[6]
                V.tensor_tensor(hA[:], u[:], rd[:], AF.mult)
                hneg = t[7]
                V.tensor_scalar(hneg[:], hA[:], 0.0, None, AF.is_lt)
                hA2 = t[8]
                # hA2 = 6*hneg + hA   (mod-6 fix: hue6 in [-1,1) -> [0,6))
                V.scalar_tensor_tensor(hA2[:], hneg[:], 6.0, hA[:], AF.mult, AF.add)
                hB = t[7]
                V.tensor_tensor(hB[:], v[:], rd[:], AF.mult)
                hC = t[6]
                V.tensor_tensor(hC[:], w[:], rd[:], AF.mult)
                mb = t[0]
                V.tensor_tensor(mb[:], mx[:], bl[:], AF.is_equal)
                mg = t[4]
                V.tensor_tensor(mg[:], mx[:], g[:], AF.is_equal)
                nmg = t[5]
                # nmg = (mb - 1) * mg = -[mg & !mb]
                V.scalar_tensor_tensor(nmg[:], mb[:], 1.0, mg[:], AF.subtract, AF.mult)
                q1 = t[9]
                V.tensor_tensor(q1[:], hB[:], hA2[:], AF.subtract)
                q2n = t[7]
                # q2n = (q1 + 2) * nmg  == -(hB - hA2 + 2)*mg'
                V.scalar_tensor_tensor(q2n[:], q1[:], 2.0, nmg[:], AF.add, AF.mult)
                h1 = t[9]
                V.tensor_tensor(h1[:], hA2[:], q2n[:], AF.subtract)
                q3 = t[4]
                V.tensor_tensor(q3[:], hC[:], hA2[:], AF.subtract)
                q4 = t[6]
                V.scalar_tensor_tensor(q4[:], q3[:], 4.0, mb[:], AF.add, AF.mult)
                hue6 = t[8]
                V.tensor_tensor(hue6[:], h1[:], q4[:], AF.add)

                acc = io_pool.tile([P, ACCW], F32, tag="acc")
                scr = t[5]
                scr2 = tmp_pool.tile([P, F], F32, tag="scr2", name="scr2")
                scr3 = tmp_pool.tile([P, F], F32, tag="scr3", name="scr3")
                SIGN = mybir.ActivationFunctionType.Sign
                # all masks on ScalarE, sign-style:
                # accum = sum(Sign(x - e)); host decodes C_lt = (N - S)/2
                for k in range(1, 10):
                    nc.scalar.activation(
                        scr2[:], hue6[:], SIGN, bias=hbias[k - 1][:],
                        accum_out=acc[:, k - 1 : k],
                    )
                # sat/val: sign-style counts on ScalarE:
                # accum = sum(Sign(x - e)); host decodes C_lt = (N - S)/2
                for k in range(1, 10):
                    nc.scalar.activation(
                        scr[:], sat[:], SIGN, bias=ebias[k - 1][:],
                        accum_out=acc[:, 9 + k - 1 : 9 + k],
                    )
                for k in range(1, 10):
                    nc.scalar.activation(
                        scr3[:], mx[:], SIGN, bias=ebias[k - 1][:],
                        accum_out=acc[:, 18 + k - 1 : 18 + k],
                    )
                nc.sync.dma_start(
                    out[it * P : (it + 1) * P, :NEDGE], acc[:, :NEDGE]
                )

    nc.compile()
    return nc


def _register_ntff_hook():
    """Register the axon NTFF profiling hook (the container's antenv stub
    lacks axon_hooks, so trn_boot's registration was skipped). Also keep
    profile artifacts local instead of uploading to a share."""
    import types

    import antenv

    if "antenv.axon_hooks" not in sys.modules:
        mod = types.ModuleType("antenv.axon_hooks")
        holder = [None]
        mod.set_axon_ntff_profile_hook = lambda h: holder.__setitem__(0, h)
        mod.get_axon_ntff_profile_hook = lambda: holder[0]
        sys.modules["antenv.axon_hooks"] = mod
        antenv.axon_hooks = mod
    from antenv import axon_hooks

    if axon_hooks.get_axon_ntff_profile_hook() is None:
        from trn_agent_boot.trn_boot import _ntff_profile_via_ctypes

        axon_hooks.set_axon_ntff_profile_hook(
            _ntff_profile_via_ctypes("/opt/axon/libaxon_pjrt.so")
        )
    bass_utils.upload_artifacts = lambda tmpdir: tmpdir


def _get_nc():
    if "nc" not in _CACHE:
        _CACHE["nc"] = _build()
    return _CACHE["nc"]


def kernel(x_real: np.ndarray, x_fake: np.ndarray) -> np.ndarray:
    global LAST_EXEC_NS
    nc = _get_nc()

    in_maps = []
    for c in range(NCORES):
        sl = slice(c * BPC, (c + 1) * BPC)
        in_maps.append(
            {
                "x_real": np.ascontiguousarray(x_real[sl]).reshape(BPC * C * P, F),
                "x_fake": np.ascontiguousarray(x_fake[sl]).reshape(BPC * C * P, F),
            }
        )

    import os

    trace = bool(int(os.environ.get("KERNEL_TRACE", "0")))
    if trace:
        _register_ntff_hook()
    res = bass_utils.run_bass_kernel_spmd(
        nc, in_maps, core_ids=list(range(NCORES)), trace=trace
    )
    LAST_EXEC_NS = res.exec_time_ns
    _CACHE["last_res"] = res

    # All 27 slots are sign-sums S = cnt_gt - cnt_lt per edge; decode as
    # C_lt[t, comp, k] = (N - S)/2 (exact-equality pixels are half-counted;
    # measure-zero for this data).
    C_lt = np.zeros((2, 3, 9), np.float64)
    for core_out in res.results:
        o = np.asarray(core_out["out"]).reshape(NITER, P, ACCW)
        s = o[:, :, :NEDGE].sum(axis=1)          # [NITER, 27]
        C_lt[0] += s[:BPC].sum(axis=0).reshape(3, 9)
        C_lt[1] += s[BPC:].sum(axis=0).reshape(3, 9)
    C_lt = (NPIX - C_lt) / 2.0

    hist = np.zeros((2, 3, 10), np.float64)
    hist[:, :, 0] = C_lt[:, :, 0]
    hist[:, :, 1:9] = C_lt[:, :, 1:] - C_lt[:, :, :-1]
    hist[:, :, 9] = NPIX - C_lt[:, :, 8]

    dmean = np.abs(hist[0] - hist[1]).mean(axis=1)   # [3] = h, s, v
    loss = ALPHA * dmean[0] + BETA * dmean[1] + GAMMA * dmean[2]
    return np.asarray(loss, dtype=np.float32)


# revision 21
# speedup vs baseline: 1.9381x; 1.1685x over previous
"""ColorHistogramLoss Trainium2 kernel (8 NeuronCores, data-parallel).

Strategy: shard batch (32 -> 4 per core). Each core streams its 25MB of
pixels through SBUF in [128, 2048] plane-tiles, computes HSV per pixel on
VectorE (fp32), and produces cumulative histogram-edge counts
C(e) = #{x < e} for 9 edges x 3 components x (real, fake) via fused
tensor_scalar(is_lt, accum_out=...) ops.  Per-(iteration, edge)
per-partition counts are DMA'd out ([8*128, 32] per core); the host sums
partitions/cores, differences cumulative counts into 10-bin histograms and
computes the scalar loss.  All on-device count arithmetic is exact in f32.
"""

import sys

if "/opt/trn_rl_repo" not in sys.path:
    sys.path.insert(0, "/opt/trn_rl_repo")

import numpy as np

from concourse import bacc, mybir, tile
from concourse import bass_utils

# ---- problem constants (hardcoded; kernel.py must be self-contained) ----
B, C, H, W = 32, 3, 512, 512
NCORES = 8
BPC = B // NCORES            # batches per core
P, F = 128, 2048             # SBUF tile: one [512,512] plane = [128, 2048]
NITER = 2 * BPC              # 4 real + 4 fake plane-triple iterations
NEDGE = 27                   # 9 edges x (hue, sat, val)
ACCW = 32                    # padded accumulator width
NPIX = B * H * W             # pixels per full histogram
ALPHA, BETA, GAMMA = 0.3, 0.4, 0.4

AF = mybir.AluOpType
F32 = mybir.dt.float32

LAST_EXEC_NS = None
_CACHE = {}

PACK = 4096.0  # EDGE2* dual-count packing: accum = cntA + PACK*cntB (exact in f32)


def _register_custom_ops():
    """Author + register fused DVE ops in the dve_ops registry at runtime
    (the repo list is read-only; registration is by-name so appending to the
    module-level OPS list is sufficient for table-gen and tracing)."""
    from concourse import dve_ops
    from concourse.dve_spec import (
        C0, C1, C2, Spec, Src0, Src1, Zero, _has_src1, lower, maxx,
    )
    from concourse.dve_uop import DveOpSpec

    if hasattr(dve_ops, "HUE_MOD6"):
        return dve_ops

    _y = Src0 * Src1

    def _ref_hue_mod6(in0, in1, c0, c1, c2):
        y = in0.astype(np.float32) * in1
        return (y + c0 * (y < 0)).astype(np.float32)

    def _ref_abs2max(in0, in1, c0, c1, c2):
        return np.maximum(np.abs(in0.astype(np.float32)), np.abs(in1)).astype(
            np.float32
        )

    def _ref_absmax3(in0, in1, c0, c1, c2):
        return np.maximum(in0.astype(np.float32), np.abs(in1)).astype(np.float32)

    def _ref_edge2d(in0, in1, c0, c1, c2):
        b = ((in0.astype(np.float32) * c0 > in1) + c1 * (in0 * c2 > in1)).astype(
            np.float32
        )
        return b, b.reshape(b.shape[0], -1).sum(axis=-1, keepdims=True)

    from operator import add as _add

    defs = [
        # out = y + c0*(y<0), y = in0*in1   (hue mod-6 wrap, fused)
        ("HUE_MOD6", Spec(body=_y + C0 * (_y < Zero), reference=_ref_hue_mod6)),
        # out = max(|in0|, |in1|)
        (
            "ABS2MAX",
            Spec(
                body=maxx(maxx(Src0, Zero - Src0), maxx(Src1, Zero - Src1)),
                reference=_ref_abs2max,
            ),
        ),
        # out = max(in0, |in1|)
        (
            "ABSMAX3",
            Spec(
                body=maxx(Src0, maxx(Src1, Zero - Src1)),
                reference=_ref_absmax3,
            ),
        ),
        # dual sat-edge count: accum = #{in0*c0 > in1} + c1*#{in0*c2 > in1}
        (
            "EDGE2D",
            Spec(
                body=(Src0 * C0 > Src1) + C1 * ((Src0 * C2) > Src1),
                accum=_add,
                accum_init=Zero,
                reference=_ref_edge2d,
            ),
        ),
    ]
    for name, spec in defs:
        row = 1 + len(dve_ops.OPS)
        shas = {}
        for ver in ("v3", "v4"):
            uops = lower(spec, ver=ver)
            shas[ver] = DveOpSpec(
                name=name, opcode=row, uops=uops, rd1_en=_has_src1(spec)
            ).sha(ver)
        op = dve_ops.DveOp(name, spec, False, uops_sha=shas)
        dve_ops.OPS.append(op)
        dve_ops.CUSTOM_DVE_SPECS[name] = spec
        dve_ops._SUB_OPCODE_FOR_NAME[name] = row
        setattr(dve_ops, name, op)
    return dve_ops


def _build():
    dve_ops = _register_custom_ops()
    nc = bacc.Bacc(
        "TRN2", target_bir_lowering=False, debug=False, num_devices=NCORES
    )
    xr = nc.dram_tensor("x_real", [BPC * C * P, F], F32, kind="ExternalInput").ap()
    xf = nc.dram_tensor("x_fake", [BPC * C * P, F], F32, kind="ExternalInput").ap()
    out = nc.dram_tensor("out", [NITER * P, ACCW], F32, kind="ExternalOutput").ap()

    with tile.TileContext(nc) as tc:
        with tc.tile_pool(name="main", bufs=2) as io_pool, tc.tile_pool(
            name="tmp", bufs=1
        ) as tmp_pool:
            # per-edge bias tiles for ScalarE Sign activations
            ebias, hbias = [], []
            for k in range(1, 10):
                bt = tmp_pool.tile([P, 1], F32, tag=f"eb{k}", name=f"eb{k}")
                nc.gpsimd.memset(bt[:], -(0.1 * k))
                ebias.append(bt)
                ht = tmp_pool.tile([P, 1], F32, tag=f"hb{k}", name=f"hb{k}")
                nc.gpsimd.memset(ht[:], -(0.6 * k))
                hbias.append(ht)
            for it in range(NITER):
                src = xr if it < BPC else xf
                bi = it % BPC

                def plane(c):
                    q = bi * C + c
                    return src[q * P : (q + 1) * P, :]

                r = io_pool.tile([P, F], F32, tag="r")
                g = io_pool.tile([P, F], F32, tag="g")
                bl = io_pool.tile([P, F], F32, tag="bl")
                nc.sync.dma_start(r[:], plane(0))
                nc.sync.dma_start(g[:], plane(1))
                nc.sync.dma_start(bl[:], plane(2))

                t = [
                    tmp_pool.tile([P, F], F32, tag=f"t{i}", name=f"t{i}")
                    for i in range(12)
                ]
                V = nc.vector

                m1, mx = t[0], t[1]
                V.tensor_tensor(m1[:], r[:], g[:], AF.max)
                V.tensor_tensor(mx[:], m1[:], bl[:], AF.max)
                u = t[2]
                V.tensor_tensor(u[:], g[:], bl[:], AF.subtract)
                v = t[4]
                V.tensor_tensor(v[:], bl[:], r[:], AF.subtract)
                w = t[5]
                V.tensor_tensor(w[:], r[:], g[:], AF.subtract)
                # d = mx - mn == max(|u|, |v|, |w|) (exact: same fl-subtracts)
                d2 = t[3]
                V._custom_dve(dve_ops.ABS2MAX, out=d2[:], in0=u[:], in1=v[:])
                d = t[10]
                V._custom_dve(dve_ops.ABSMAX3, out=d[:], in0=d2[:], in1=w[:])
                rd = t[3]
                V.reciprocal_approx_fast(rd[:], d[:])
                hA2 = t[8]
                # hA2 = u*rd + 6*(u*rd < 0)   (hue mod-6 wrap, one fused op)
                V._custom_dve(
                    dve_ops.HUE_MOD6, out=hA2[:], in0=u[:], in1=rd[:], s0=6.0
                )
                hB = t[7]
                V.tensor_tensor(hB[:], v[:], rd[:], AF.mult)
                hC = t[6]
                V.tensor_tensor(hC[:], w[:], rd[:], AF.mult)
                mb = t[0]
                V.tensor_tensor(mb[:], mx[:], bl[:], AF.is_equal)
                mg = t[4]
                V.tensor_tensor(mg[:], mx[:], g[:], AF.is_equal)
                nmg = t[5]
                # nmg = (mb - 1) * mg = -[mg & !mb]
                V.scalar_tensor_tensor(nmg[:], mb[:], 1.0, mg[:], AF.subtract, AF.mult)
                q1 = t[9]
                V.tensor_tensor(q1[:], hB[:], hA2[:], AF.subtract)
                q2n = t[7]
                # q2n = (q1 + 2) * nmg  == -(hB - hA2 + 2)*mg'
                V.scalar_tensor_tensor(q2n[:], q1[:], 2.0, nmg[:], AF.add, AF.mult)
                h1 = t[9]
                V.tensor_tensor(h1[:], hA2[:], q2n[:], AF.subtract)
                q3 = t[4]
                V.tensor_tensor(q3[:], hC[:], hA2[:], AF.subtract)
                q4 = t[6]
                V.scalar_tensor_tensor(q4[:], q3[:], 4.0, mb[:], AF.add, AF.mult)
                hue6 = t[8]
                V.tensor_tensor(hue6[:], h1[:], q4[:], AF.add)

                acc = io_pool.tile([P, ACCW], F32, tag="acc")
                scr = t[5]
                scr2 = tmp_pool.tile([P, F], F32, tag="scr2", name="scr2")
                scr3 = tmp_pool.tile([P, F], F32, tag="scr3", name="scr3")
                SIGN = mybir.ActivationFunctionType.Sign
                # all masks on ScalarE, sign-style:
                # accum = sum(Sign(x - e)); host decodes C_lt = (N - S)/2
                for k in range(1, 10):
                    nc.scalar.activation(
                        scr2[:], hue6[:], SIGN, bias=hbias[k - 1][:],
                        accum_out=acc[:, k - 1 : k],
                    )
                # sat/val: sign-style counts on ScalarE:
                # accum = sum(Sign(x - e)); host decodes C_lt = (N - S)/2
                for k in range(1, 10):
                    nc.scalar.activation(
                        scr3[:], mx[:], SIGN, bias=ebias[k - 1][:],
                        accum_out=acc[:, 9 + k - 1 : 9 + k],
                    )
                # sat masks on VectorE: dual-edge fused counts.
                # slot = #{0.1(2j+1)*mx > d} + PACK * #{0.1(2j+2)*mx > d}
                for j in range(4):
                    V._custom_dve(
                        dve_ops.EDGE2D,
                        out=scr[:],
                        in0=mx[:],
                        in1=d[:],
                        s0=0.1 * (2 * j + 1),
                        s1=PACK,
                        imm2=0.1 * (2 * j + 2),
                        accum_out=acc[:, 18 + j : 19 + j],
                    )
                # 9th sat edge: direct single count
                V.scalar_tensor_tensor(
                    scr[:], mx[:], 0.9, d[:], AF.mult, AF.is_gt,
                    accum_out=acc[:, 22:23],
                )
                nc.sync.dma_start(
                    out[it * P : (it + 1) * P, :NEDGE], acc[:, :NEDGE]
                )

    nc.compile()
    return nc


def _register_ntff_hook():
    """Register the axon NTFF profiling hook (the container's antenv stub
    lacks axon_hooks, so trn_boot's registration was skipped). Also keep
    profile artifacts local instead of uploading to a share."""
    import types

    import antenv

    if "antenv.axon_hooks" not in sys.modules:
        mod = types.ModuleType("antenv.axon_hooks")
        holder = [None]
        mod.set_axon_ntff_profile_hook = lambda h: holder.__setitem__(0, h)
        mod.get_axon_ntff_profile_hook = lambda: holder[0]
        sys.modules["antenv.axon_hooks"] = mod
        antenv.axon_hooks = mod
    from antenv import axon_hooks

    if axon_hooks.get_axon_ntff_profile_hook() is None:
        from trn_agent_boot.trn_boot import _ntff_profile_via_ctypes

        axon_hooks.set_axon_ntff_profile_hook(
            _ntff_profile_via_ctypes("/opt/axon/libaxon_pjrt.so")
        )
    bass_utils.upload_artifacts = lambda tmpdir: tmpdir


def _get_nc():
    if "nc" not in _CACHE:
        _CACHE["nc"] = _build()
    return _CACHE["nc"]


def kernel(x_real: np.ndarray, x_fake: np.ndarray) -> np.ndarray:
    global LAST_EXEC_NS
    nc = _get_nc()

    in_maps = []
    for c in range(NCORES):
        sl = slice(c * BPC, (c + 1) * BPC)
        in_maps.append(
            {
                "x_real": np.ascontiguousarray(x_real[sl]).reshape(BPC * C * P, F),
                "x_fake": np.ascontiguousarray(x_fake[sl]).reshape(BPC * C * P, F),
            }
        )

    import os

    trace = bool(int(os.environ.get("KERNEL_TRACE", "0")))
    if trace:
        _register_ntff_hook()
    res = bass_utils.run_bass_kernel_spmd(
        nc, in_maps, core_ids=list(range(NCORES)), trace=trace
    )
    LAST_EXEC_NS = res.exec_time_ns
    _CACHE["last_res"] = res

    # Slots 0:9 hue, 9:18 val: sign-sums S = cnt_gt - cnt_lt, decoded as
    # C_lt = (N - S)/2.  Slots 18:22: packed dual sat counts
    # cntA + PACK*cntB (direct C_lt); slot 22: sat edge 9 direct count.
    sign_sums = np.zeros((2, 18), np.float64)
    sat_C = np.zeros((2, 9), np.float64)
    for core_out in res.results:
        o = np.asarray(core_out["out"]).reshape(NITER, P, ACCW)
        for t_idx, sl in ((0, slice(0, BPC)), (1, slice(BPC, NITER))):
            blk = o[sl]
            sign_sums[t_idx] += blk[:, :, :18].sum(axis=(0, 1))
            packed = blk[:, :, 18:22].astype(np.int64)  # exact ints in f32
            sat_C[t_idx, 0:8:2] += (packed % int(PACK)).sum(axis=(0, 1))
            sat_C[t_idx, 1:8:2] += (packed // int(PACK)).sum(axis=(0, 1))
            sat_C[t_idx, 8] += blk[:, :, 22].sum()
    C_lt = np.zeros((2, 3, 9), np.float64)
    C_lt[:, 0, :] = (NPIX - sign_sums[:, 0:9]) / 2.0   # hue
    C_lt[:, 2, :] = (NPIX - sign_sums[:, 9:18]) / 2.0  # val
    C_lt[:, 1, :] = sat_C

    hist = np.zeros((2, 3, 10), np.float64)
    hist[:, :, 0] = C_lt[:, :, 0]
    hist[:, :, 1:9] = C_lt[:, :, 1:] - C_lt[:, :, :-1]
    hist[:, :, 9] = NPIX - C_lt[:, :, 8]

    dmean = np.abs(hist[0] - hist[1]).mean(axis=1)   # [3] = h, s, v
    loss = ALPHA * dmean[0] + BETA * dmean[1] + GAMMA * dmean[2]
    return np.asarray(loss, dtype=np.float32)
